# revision 22
# baseline (speedup 1.0000x reference)
"""Trainium2 Bass kernel for the CustomGNNLayer problem.

Strategy (data-parallel over Q, 8 queries/core on 8 cores):
  host: compute the tiny projection heads (rel softmax + prob gather, gq,
        c_q) in numpy; compact node slots per (q,k) group (drop all-zero
        padded slots; pad kept counts to PAD_MULT with a per-block class
        profile uniform across cores so one SPMD program fits all cores,
        then pad each block to a 16-slot multiple); nodes+Wn in fp8e4,
        everything big else bf16; fold mask / mean divisors into maskf.
  device (per core, per (q,k) block): X = Wn^T @ nodesT in one fp8
        DoubleRow matmul per (d-tile, piece); tanh+bias on ScalarE; dots
        via per-d-tile PE matmuls with gq as 1-column stationary weights,
        accumulated in PSUM; group softmax + global softmax on [N,M] grid;
        weighted sum of nodes via PE-broadcast wa + fused DVE
        multiply-reduce; final tanh projection -> updated rows.
  host: res = hidden_states.copy(); res[gnn_idx] += rows.
"""
import sys

sys.path.insert(0, "/opt/trn_rl_repo")

import numpy as np
import ml_dtypes

import concourse.bacc as bacc
import concourse.bass as bass
import concourse.tile as tile
from concourse import mybir
from concourse.bass_utils import run_bass_kernel_spmd

F32 = mybir.dt.float32
BF16 = mybir.dt.bfloat16
FP8 = mybir.dt.float8e4
AF = mybir.ActivationFunctionType
ALU = mybir.AluOpType
AX = mybir.AxisListType
DR = mybir.MatmulPerfMode.DoubleRow

Q, K, N, M = 64, 2, 32, 64
E, D, R, S = 256, 1024, 200, 8192
NCORES = 8
QPC = Q // NCORES          # 8 queries per core
NB = QPC * K               # 16 blocks per core, b = qi*K + k
PAD_MULT = 8
ET = E // 128              # 2 e-tiles
DT = D // 128              # 8 d-tiles
PSW = 512                  # psum bank width (f32)
ACTW = 3 * PSW             # activation span (3 psum banks)

BF16NP = ml_dtypes.bfloat16
FP8NP = ml_dtypes.float8_e4m3fn


def _pieces(size, step=PSW):
    return [(p0, min(step, size - p0)) for p0 in range(0, size, step)]


def _host_prep(inputs):
    hs = np.ascontiguousarray(inputs["hidden_states"], dtype=np.float32)
    nodes = np.ascontiguousarray(inputs["nodes"], dtype=np.float32)
    prob_idx = np.asarray(inputs["prob_idx"])
    gnn_idx = np.asarray(inputs["gnn_idx"]).astype(np.int64)
    rel_idx = np.asarray(inputs["rel_idx"]).astype(np.int64)
    Wc = np.asarray(inputs["Wc"], np.float32); bc = np.asarray(inputs["bc"], np.float32)
    Wq = np.asarray(inputs["Wq"], np.float32); bq = np.asarray(inputs["bq"], np.float32)
    Wn = np.asarray(inputs["Wn"], np.float32); bn = np.asarray(inputs["bn"], np.float32)
    Wg = np.asarray(inputs["Wg"], np.float32); bg = np.asarray(inputs["bg"], np.float32)

    # tiny projection heads on host
    rl = hs[rel_idx] @ Wc + bc                          # [Q,R]
    rl -= rl.max(axis=1, keepdims=True)
    np.exp(rl, out=rl)
    rel_prob = rl / rl.sum(axis=1, keepdims=True)
    probs10 = 10.0 * np.take_along_axis(
        rel_prob, prob_idx.reshape(Q, K * N), axis=1).reshape(Q, K, N)
    gq = np.tanh(hs[gnn_idx] @ Wq + bq)                 # [Q,D]
    cq = gq @ np.tanh(bn)                               # [Q]

    nz = np.any(nodes != 0.0, axis=4)          # [Q,K,N,M] kept slots
    lens = nz.sum(axis=3)                      # [Q,K,N]
    Lg = np.minimum(((np.maximum(lens, 1) + PAD_MULT - 1) // PAD_MULT) * PAD_MULT, M)

    # per-block-index profile: position-wise max of descending-sorted Lg across cores
    profiles = []   # [NB][N] descending class sizes, uniform across cores
    for qi in range(QPC):
        for k in range(K):
            seqs = [np.sort(Lg[c * QPC + qi, k])[::-1] for c in range(NCORES)]
            profiles.append(np.max(np.stack(seqs), axis=0))
    S_raw = [int(p.sum()) for p in profiles]
    S_b = [((s + 15) // 16) * 16 for s in S_raw]   # pad to 16 for fp8 APs
    segs = []       # [NB] list of (L, row0, cnt, slot_off)
    for p in profiles:
        s, off = [], 0
        i = 0
        while i < N:
            j = i
            while j < N and p[j] == p[i]:
                j += 1
            L = int(p[i])
            s.append((L, i, j - i, off))
            off += L * (j - i)
            i = j
        segs.append(s)

    mask0 = (nodes[..., 0] != 0.0)             # [Q,K,N,M] reference mask

    per_core = []
    for c in range(NCORES):
        qs = np.arange(c * QPC, (c + 1) * QPC)
        nt_flat = np.empty(sum(2 * 128 * s for s in S_b), FP8NP)
        maskf = np.zeros((NB, N, M), np.float32)
        spr = np.zeros((N, NB), np.float32)
        ntoff = 0
        for qi in range(QPC):
            q = qs[qi]
            for k in range(K):
                b = qi * K + k
                prof = profiles[b]
                order = np.argsort(-Lg[q, k], kind="stable")   # ranks -> groups
                comp = np.zeros((S_b[b], E), np.float32)
                off = 0
                for rank, g in enumerate(order):
                    L = int(prof[rank])
                    keep = np.nonzero(nz[q, k, g])[0]
                    nkeep = len(keep)
                    comp[off : off + nkeep] = nodes[q, k, g, keep]
                    maskf[b, rank, :nkeep] = mask0[q, k, g, keep].astype(np.float32)
                    spr[rank, b] = probs10[q, k, g]
                    off += L
                nt = comp.T.astype(FP8NP)                      # [E, S_b] fp8
                sz = 2 * 128 * S_b[b]
                nt_flat[ntoff : ntoff + sz] = nt.reshape(-1)
                ntoff += sz
        maskf *= 1.0 / (N * M * K)
        # gqT [128, DT, QPC]: d = t*128 + p
        gqT = np.ascontiguousarray(
            gq[qs].reshape(QPC, DT, 128).transpose(2, 1, 0)).astype(BF16NP)
        scq = np.ascontiguousarray(
            np.broadcast_to(cq[qs][None, :], (N, QPC)), np.float32)
        per_core.append({
            "nodesT": nt_flat,
            "maskf": maskf,
            "spr": spr,
            "sgq": gqT,
            "scq": scq,
        })

    shared = {
        "Wn": Wn.astype(FP8NP),
        "Wg": Wg.astype(BF16NP),
        "bn": np.ascontiguousarray(bn.reshape(DT, 128).T),
        "bg": np.ascontiguousarray(bg.reshape(DT, 128).T),
        "ones128": np.ones((1, 128), BF16NP),
    }
    for pc in per_core:
        pc.update(shared)
    return per_core, S_b, S_raw, segs, gnn_idx, hs


def _build_program(S_b, S_raw, segs):
    nc = bacc.Bacc("TRN2", target_bir_lowering=False, debug=False,
                   num_devices=NCORES)
    S_MAX = max(S_b)
    NT_TOT = sum(2 * 128 * s for s in S_b)

    d_nodesT = nc.dram_tensor("nodesT", [NT_TOT], FP8, kind="ExternalInput").ap()
    d_Wn = nc.dram_tensor("Wn", [E, D], FP8, kind="ExternalInput").ap()
    d_Wg = nc.dram_tensor("Wg", [E, D], BF16, kind="ExternalInput").ap()
    d_bn = nc.dram_tensor("bn", [128, DT], F32, kind="ExternalInput").ap()
    d_bg = nc.dram_tensor("bg", [128, DT], F32, kind="ExternalInput").ap()
    d_ones = nc.dram_tensor("ones128", [1, 128], BF16, kind="ExternalInput").ap()
    d_maskf = nc.dram_tensor("maskf", [NB, N, M], F32, kind="ExternalInput").ap()
    d_spr = nc.dram_tensor("spr", [N, NB], F32, kind="ExternalInput").ap()
    d_sgq = nc.dram_tensor("sgq", [128, DT, QPC], BF16, kind="ExternalInput").ap()
    d_scq = nc.dram_tensor("scq", [N, QPC], F32, kind="ExternalInput").ap()
    d_outT = nc.dram_tensor("outT", [D, QPC], F32, kind="ExternalOutput").ap()

    # DRAM scratch
    d_dots = nc.dram_tensor("sc_dots", [NB, 2048], F32).ap()
    d_wa = nc.dram_tensor("sc_wa", [NB, 2048], BF16).ap()
    d_ginv = nc.dram_tensor("sc_ginv", [NB, 1], F32).ap()

    with tile.TileContext(nc) as tc:
        with tc.tile_pool(name="wts", bufs=1) as wts, \
             tc.tile_pool(name="big", bufs=4) as big, \
             tc.tile_pool(name="zp", bufs=10) as zp, \
             tc.tile_pool(name="sml", bufs=6) as sml, \
             tc.tile_pool(name="mep", bufs=24) as mep, \
             tc.tile_pool(name="jnk", bufs=2) as jnk, \
             tc.tile_pool(name="ps", bufs=2, space="PSUM") as ps, \
             tc.tile_pool(name="pp", bufs=2, space="PSUM") as pp:

            # ---------------- load constants ----------------
            sWn = wts.tile([128, ET, D], FP8)
            nc.sync.dma_start(sWn, d_Wn.rearrange("(t p) r -> p t r", p=128))
            sWg = wts.tile([128, ET, D], BF16)
            nc.sync.dma_start(sWg, d_Wg.rearrange("(t p) r -> p t r", p=128))
            sbn = wts.tile([128, DT], F32)
            nc.sync.dma_start(sbn, d_bn)
            sbg = wts.tile([128, DT], F32)
            nc.sync.dma_start(sbg, d_bg)
            sones_r = wts.tile([1, 128], BF16)
            nc.sync.dma_start(sones_r, d_ones)
            smaskf = wts.tile([N, NB, M], F32)
            nc.sync.dma_start(smaskf, d_maskf.rearrange("b n m -> n b m"))
            spr = wts.tile([N, NB], F32)
            nc.sync.dma_start(spr, d_spr)
            sgq = wts.tile([128, DT, QPC], BF16)
            nc.sync.dma_start(sgq, d_sgq)
            scq = wts.tile([N, QPC], F32)
            nc.sync.dma_start(scq, d_scq)
            zpad = wts.tile([1, 16], BF16)
            nc.vector.memset(zpad, 0.0)

            # ---------------- main loop ----------------
            t_pooled = wts.tile([128, ET, QPC], F32)
            nt_off = 0
            from collections import defaultdict
            partials = defaultdict(list)
            for b in range(NB):
                qi, k = b // K, b % K
                sb = S_b[b]

                t_nt = big.tile([128, ET, S_MAX], FP8, tag="nt")
                nc.sync.dma_start(
                    t_nt[:, :, 0:sb],
                    bass.AP(tensor=d_nodesT.tensor, offset=nt_off,
                            ap=[[sb, 128], [128 * sb, ET], [1, sb]]))
                nt_off += 2 * 128 * sb

                # ---- X = Wn^T @ nt (fp8 DoubleRow), tanh -> z, dots via PE ----
                t_dots = sml.tile([1, S_MAX], F32, tag="dots")
                for a0 in range(0, sb, ACTW):
                    aw = min(ACTW, sb - a0)
                    zs = []
                    for dt_i in range(DT):
                        p_x = ps.tile([128, ACTW], F32, tag="mm")
                        for (p0, pw) in _pieces(aw):
                            nc.tensor.matmul(
                                p_x[:, p0 : p0 + pw],
                                sWn[:, :, dt_i * 128:(dt_i + 1) * 128],
                                t_nt[:, :, a0 + p0 : a0 + p0 + pw],
                                start=True, stop=True, perf_mode=DR)
                        t_z = zp.tile([128, ACTW], BF16, tag="z")
                        nc.scalar.activation(t_z[:, 0:aw], p_x[:, 0:aw],
                                             AF.Tanh, bias=sbn[:, dt_i : dt_i + 1],
                                             scale=1.0)
                        zs.append(t_z)
                    for (p0, pw) in _pieces(aw):
                        p_d = pp.tile([128, PSW], F32, tag="pp")
                        for dt_i in range(DT):
                            nc.tensor.matmul(p_d[0:1, 0:pw],
                                             sgq[:, dt_i, qi : qi + 1],
                                             zs[dt_i][:, p0 : p0 + pw],
                                             start=(dt_i == 0),
                                             stop=(dt_i == DT - 1))
                        nc.vector.tensor_copy(t_dots[0:1, a0 + p0 : a0 + p0 + pw],
                                              p_d[0:1, 0:pw])
                nc.sync.dma_start(d_dots[b : b + 1, 0:sb], t_dots[0:1, 0:sb])

                # ---- softmax on [N, M] grid ----
                t_dbuf = sml.tile([N, M], F32, tag="dbuf")
                nc.gpsimd.tensor_scalar(t_dbuf, smaskf[:, b, :], 0.0,
                                        scq[:, qi : qi + 1],
                                        op0=ALU.mult, op1=ALU.add)
                for (L, r0, cnt, soff) in segs[b]:
                    nc.gpsimd.dma_start(
                        t_dbuf[r0 : r0 + cnt, 0:L],
                        d_dots[b, soff : soff + cnt * L].rearrange("(c l) -> c l", l=L))

                t_gmx = sml.tile([N, 1], F32, tag="gmx")
                nc.vector.tensor_reduce(t_gmx, t_dbuf, axis=AX.X, op=ALU.max)
                t_gnmx = sml.tile([N, 1], F32, tag="gnmx")
                nc.gpsimd.tensor_scalar_mul(t_gnmx, t_gmx, -1.0)
                t_ex = sml.tile([N, M], F32, tag="ex")
                t_rs = sml.tile([N, 1], F32, tag="rs")
                nc.scalar.activation(t_ex, t_dbuf, AF.Exp, bias=t_gnmx, scale=1.0,
                                     accum_out=t_rs)
                t_ri = sml.tile([N, 1], F32, tag="ri")
                nc.vector.reciprocal(t_ri, t_rs)
                t_lg = sml.tile([N, M], F32, tag="lg")
                nc.vector.tensor_scalar(t_lg, t_ex, t_ri, spr[:, b : b + 1],
                                        op0=ALU.mult, op1=ALU.mult)
                t_gl = sml.tile([N, M], F32, tag="gl")
                t_grs = sml.tile([N, 1], F32, tag="grs")
                nc.scalar.activation(t_gl, t_lg, AF.Exp, accum_out=t_grs)
                t_gs = sml.tile([1, 1], F32, tag="gs")
                nc.gpsimd.tensor_reduce(t_gs, t_grs, axis=AX.C, op=ALU.add)
                t_gi = sml.tile([1, 1], F32, tag="gi")
                nc.vector.reciprocal(t_gi, t_gs)
                nc.sync.dma_start(d_ginv[b : b + 1, :], t_gi)
                t_gic = sml.tile([N, 1], F32, tag="gic")
                nc.sync.dma_start(
                    t_gic,
                    bass.AP(tensor=d_ginv.tensor, offset=b, ap=[[0, N], [1, 1]]))
                t_wa = sml.tile([N, M], BF16, tag="wa")
                nc.vector.scalar_tensor_tensor(
                    t_wa, t_gl, t_gic, smaskf[:, b, :],
                    op0=ALU.mult, op1=ALU.mult)

                # gather back to compacted order (+ zero the 16-pad tail)
                for (L, r0, cnt, soff) in segs[b]:
                    nc.sync.dma_start(
                        d_wa[b, soff : soff + cnt * L].rearrange("(c l) -> c l", l=L),
                        t_wa[r0 : r0 + cnt, 0:L])
                if S_raw[b] < sb:
                    nc.gpsimd.dma_start(d_wa[b, S_raw[b] : sb],
                                        zpad[0, 0 : sb - S_raw[b]])
                t_wac = sml.tile([1, S_MAX], BF16, tag="wac")
                nc.sync.dma_start(t_wac[0:1, 0:sb], d_wa[b : b + 1, 0:sb])

                # ---- pass 2: me[e] = sum_s nodesT[e, s] * wa[s] ----
                for (p0, pw) in _pieces(sb):
                    p_w = pp.tile([128, PSW], F32, tag="pp")
                    nc.tensor.matmul(p_w[:, 0:pw], sones_r,
                                     t_wac[0:1, p0 : p0 + pw],
                                     start=True, stop=True)
                    for et in range(ET):
                        t_me = mep.tile([128, 1], F32, tag="me")
                        t_junk = jnk.tile([128, PSW], BF16, tag="junk")
                        nc.vector.scalar_tensor_tensor(
                            out=t_junk[:, 0:pw],
                            in0=t_nt[:, et, p0 : p0 + pw],
                            scalar=1.0,
                            in1=p_w[:, 0:pw],
                            op0=ALU.mult, op1=ALU.mult,
                            accum_out=t_me)
                        partials[(qi, et)].append(t_me)
                if k == K - 1:
                    for et in range(ET):
                        ps_list = partials.pop((qi, et))
                        acc_t = ps_list[0]
                        for i, t in enumerate(ps_list[1:]):
                            is_last = i == len(ps_list) - 2
                            if is_last:
                                dst = t_pooled[:, et, qi : qi + 1]
                            else:
                                dst = mep.tile([128, 1], F32, tag="macc")
                            nc.vector.tensor_tensor(dst, acc_t, t, op=ALU.add)
                            acc_t = dst

            # ---------------- output projection ----------------
            t_plr = wts.tile([128, ET, QPC], BF16)
            nc.vector.tensor_copy(t_plr, t_pooled)
            t_outT = wts.tile([128, DT, QPC], F32)
            for mt in range(DT):
                p_o = ps.tile([128, ACTW], F32, tag="mm")
                for et in range(ET):
                    nc.tensor.matmul(p_o[:, 0:QPC],
                                     sWg[:, et, mt * 128:(mt + 1) * 128],
                                     t_plr[:, et, :],
                                     start=(et == 0), stop=(et == ET - 1))
                nc.scalar.activation(t_outT[:, mt, :], p_o[:, 0:QPC], AF.Tanh,
                                     bias=sbg[:, mt : mt + 1], scale=1.0)
            nc.sync.dma_start(d_outT.rearrange("(t p) q -> p t q", p=128), t_outT)

    nc.compile()
    return nc


_CACHE = {}


def kernel(**inputs) -> np.ndarray:
    per_core, S_b, S_raw, segs, gnn_idx, hs = _host_prep(inputs)
    key = tuple(S_b)
    if key not in _CACHE:
        _CACHE[key] = _build_program(S_b, S_raw, segs)
    nc = _CACHE[key]
    res = run_bass_kernel_spmd(nc, per_core, list(range(NCORES)))
    out = hs.copy()
    for c in range(NCORES):
        rows = res.results[c]["outT"].T      # [QPC, D]
        np.add.at(out, gnn_idx[c * QPC : (c + 1) * QPC], rows)
    return out


# revision 26
# speedup vs baseline: 1.0019x; 1.0019x over previous
"""Trainium2 Bass kernel for the CustomGNNLayer problem.

Strategy (data-parallel over Q, 8 queries/core on 8 cores):
  host: compute the tiny projection heads (rel softmax + prob gather, gq,
        c_q) in numpy; compact node slots per (q,k) group (drop all-zero
        padded slots; pad kept counts to PAD_MULT with a per-block class
        profile uniform across cores so one SPMD program fits all cores,
        then pad each block to a 16-slot multiple); nodes+Wn in fp8e4,
        everything big else bf16; fold mask / mean divisors into maskf.
  device (per core, per (q,k) block): X = Wn^T @ nodesT in one fp8
        DoubleRow matmul per (d-tile, piece); tanh+bias on ScalarE; dots
        via per-d-tile PE matmuls with gq as 1-column stationary weights,
        accumulated in PSUM; group softmax + global softmax on [N,M] grid;
        weighted sum of nodes via PE-broadcast wa + fused DVE
        multiply-reduce; final tanh projection -> updated rows.
  host: res = hidden_states.copy(); res[gnn_idx] += rows.
"""
import sys

sys.path.insert(0, "/opt/trn_rl_repo")

import numpy as np
import ml_dtypes

import concourse.bacc as bacc
import concourse.bass as bass
import concourse.tile as tile
from concourse import mybir
from concourse.bass_utils import run_bass_kernel_spmd

F32 = mybir.dt.float32
BF16 = mybir.dt.bfloat16
FP8 = mybir.dt.float8e4
AF = mybir.ActivationFunctionType
ALU = mybir.AluOpType
AX = mybir.AxisListType
DR = mybir.MatmulPerfMode.DoubleRow

Q, K, N, M = 64, 2, 32, 64
E, D, R, S = 256, 1024, 200, 8192
NCORES = 8
QPC = Q // NCORES          # 8 queries per core
NB = QPC * K               # 16 blocks per core, b = qi*K + k
PAD_MULT = 8
ET = E // 128              # 2 e-tiles
DT = D // 128              # 8 d-tiles
PSW = 512                  # psum bank width (f32)
ACTW = 3 * PSW             # activation span (3 psum banks)

BF16NP = ml_dtypes.bfloat16
FP8NP = ml_dtypes.float8_e4m3fn


def _pieces(size, step=PSW):
    return [(p0, min(step, size - p0)) for p0 in range(0, size, step)]


def _host_prep(inputs):
    hs = np.ascontiguousarray(inputs["hidden_states"], dtype=np.float32)
    nodes = np.ascontiguousarray(inputs["nodes"], dtype=np.float32)
    prob_idx = np.asarray(inputs["prob_idx"])
    gnn_idx = np.asarray(inputs["gnn_idx"]).astype(np.int64)
    rel_idx = np.asarray(inputs["rel_idx"]).astype(np.int64)
    Wc = np.asarray(inputs["Wc"], np.float32); bc = np.asarray(inputs["bc"], np.float32)
    Wq = np.asarray(inputs["Wq"], np.float32); bq = np.asarray(inputs["bq"], np.float32)
    Wn = np.asarray(inputs["Wn"], np.float32); bn = np.asarray(inputs["bn"], np.float32)
    Wg = np.asarray(inputs["Wg"], np.float32); bg = np.asarray(inputs["bg"], np.float32)

    # tiny projection heads on host
    rl = hs[rel_idx] @ Wc + bc                          # [Q,R]
    rl -= rl.max(axis=1, keepdims=True)
    np.exp(rl, out=rl)
    rel_prob = rl / rl.sum(axis=1, keepdims=True)
    probs10 = 10.0 * np.take_along_axis(
        rel_prob, prob_idx.reshape(Q, K * N), axis=1).reshape(Q, K, N)
    gq = np.tanh(hs[gnn_idx] @ Wq + bq)                 # [Q,D]
    cq = gq @ np.tanh(bn)                               # [Q]

    nz = np.any(nodes != 0.0, axis=4)          # [Q,K,N,M] kept slots
    lens = nz.sum(axis=3)                      # [Q,K,N]
    Lg = np.minimum(((np.maximum(lens, 1) + PAD_MULT - 1) // PAD_MULT) * PAD_MULT, M)

    # per-block-index profile: position-wise max of descending-sorted Lg across cores
    profiles = []   # [NB][N] descending class sizes, uniform across cores
    for qi in range(QPC):
        for k in range(K):
            seqs = [np.sort(Lg[c * QPC + qi, k])[::-1] for c in range(NCORES)]
            profiles.append(np.max(np.stack(seqs), axis=0))
    S_raw = [int(p.sum()) for p in profiles]
    S_b = [((s + 15) // 16) * 16 for s in S_raw]   # pad to 16 for fp8 APs
    segs = []       # [NB] list of (L, row0, cnt, slot_off)
    for p in profiles:
        s, off = [], 0
        i = 0
        while i < N:
            j = i
            while j < N and p[j] == p[i]:
                j += 1
            L = int(p[i])
            s.append((L, i, j - i, off))
            off += L * (j - i)
            i = j
        segs.append(s)

    mask0 = (nodes[..., 0] != 0.0)             # [Q,K,N,M] reference mask

    per_core = []
    for c in range(NCORES):
        qs = np.arange(c * QPC, (c + 1) * QPC)
        nt_flat = np.empty(sum(2 * 128 * s for s in S_b), FP8NP)
        maskf = np.zeros((NB, N, M), np.float32)
        spr = np.zeros((N, NB), np.float32)
        ntoff = 0
        for qi in range(QPC):
            q = qs[qi]
            for k in range(K):
                b = qi * K + k
                prof = profiles[b]
                order = np.argsort(-Lg[q, k], kind="stable")   # ranks -> groups
                comp = np.zeros((S_b[b], E), np.float32)
                off = 0
                for rank, g in enumerate(order):
                    L = int(prof[rank])
                    keep = np.nonzero(nz[q, k, g])[0]
                    nkeep = len(keep)
                    comp[off : off + nkeep] = nodes[q, k, g, keep]
                    maskf[b, rank, :nkeep] = mask0[q, k, g, keep].astype(np.float32)
                    spr[rank, b] = probs10[q, k, g]
                    off += L
                nt = comp.T.astype(FP8NP)                      # [E, S_b] fp8
                sz = 2 * 128 * S_b[b]
                nt_flat[ntoff : ntoff + sz] = nt.reshape(-1)
                ntoff += sz
        maskf *= 1.0 / (N * M * K)
        # gqT [128, DT, QPC]: d = t*128 + p
        gqT = np.ascontiguousarray(
            gq[qs].reshape(QPC, DT, 128).transpose(2, 1, 0)).astype(BF16NP)
        scq = np.ascontiguousarray(
            np.broadcast_to(cq[qs][None, :], (N, QPC)), np.float32)
        per_core.append({
            "nodesT": nt_flat,
            "maskf": maskf,
            "spr": spr,
            "sgq": gqT,
            "scq": scq,
        })

    shared = {
        "Wn": Wn.astype(FP8NP),
        "Wg": Wg.astype(BF16NP),
        "bn": np.ascontiguousarray(bn.reshape(DT, 128).T),
        "bg": np.ascontiguousarray(bg.reshape(DT, 128).T),
        "ones128": np.ones((1, 128), BF16NP),
    }
    for pc in per_core:
        pc.update(shared)
    return per_core, S_b, S_raw, segs, gnn_idx, hs


def _build_program(S_b, S_raw, segs):
    nc = bacc.Bacc("TRN2", target_bir_lowering=False, debug=False,
                   num_devices=NCORES)
    S_MAX = max(S_b)
    NT_TOT = sum(2 * 128 * s for s in S_b)

    d_nodesT = nc.dram_tensor("nodesT", [NT_TOT], FP8, kind="ExternalInput").ap()
    d_Wn = nc.dram_tensor("Wn", [E, D], FP8, kind="ExternalInput").ap()
    d_Wg = nc.dram_tensor("Wg", [E, D], BF16, kind="ExternalInput").ap()
    d_bn = nc.dram_tensor("bn", [128, DT], F32, kind="ExternalInput").ap()
    d_bg = nc.dram_tensor("bg", [128, DT], F32, kind="ExternalInput").ap()
    d_ones = nc.dram_tensor("ones128", [1, 128], BF16, kind="ExternalInput").ap()
    d_maskf = nc.dram_tensor("maskf", [NB, N, M], F32, kind="ExternalInput").ap()
    d_spr = nc.dram_tensor("spr", [N, NB], F32, kind="ExternalInput").ap()
    d_sgq = nc.dram_tensor("sgq", [128, DT, QPC], BF16, kind="ExternalInput").ap()
    d_scq = nc.dram_tensor("scq", [N, QPC], F32, kind="ExternalInput").ap()
    d_outT = nc.dram_tensor("outT", [D, QPC], F32, kind="ExternalOutput").ap()

    # DRAM scratch
    d_dots = nc.dram_tensor("sc_dots", [NB, 2048], F32).ap()
    d_wa = nc.dram_tensor("sc_wa", [NB, 2048], BF16).ap()
    d_ginv = nc.dram_tensor("sc_ginv", [NB, 1], F32).ap()

    with tile.TileContext(nc) as tc:
        with tc.tile_pool(name="wts", bufs=1) as wts, \
             tc.tile_pool(name="big", bufs=4) as big, \
             tc.tile_pool(name="zp", bufs=20) as zp, \
             tc.tile_pool(name="sml", bufs=6) as sml, \
             tc.tile_pool(name="mep", bufs=24) as mep, \
             tc.tile_pool(name="jnk", bufs=2) as jnk, \
             tc.tile_pool(name="ps", bufs=2, space="PSUM") as ps, \
             tc.tile_pool(name="pp", bufs=2, space="PSUM") as pp:

            # ---------------- load constants ----------------
            sWn = wts.tile([128, ET, D], FP8)
            nc.sync.dma_start(sWn, d_Wn.rearrange("(t p) r -> p t r", p=128))
            sWg = wts.tile([128, ET, D], BF16)
            nc.sync.dma_start(sWg, d_Wg.rearrange("(t p) r -> p t r", p=128))
            sbn = wts.tile([128, DT], F32)
            nc.sync.dma_start(sbn, d_bn)
            sbg = wts.tile([128, DT], F32)
            nc.sync.dma_start(sbg, d_bg)
            sones_r = wts.tile([1, 128], BF16)
            nc.sync.dma_start(sones_r, d_ones)
            smaskf = wts.tile([N, NB, M], F32)
            nc.sync.dma_start(smaskf, d_maskf.rearrange("b n m -> n b m"))
            spr = wts.tile([N, NB], F32)
            nc.sync.dma_start(spr, d_spr)
            sgq = wts.tile([128, DT, QPC], BF16)
            nc.sync.dma_start(sgq, d_sgq)
            scq = wts.tile([N, QPC], F32)
            nc.sync.dma_start(scq, d_scq)
            zpad = wts.tile([1, 16], BF16)
            nc.vector.memset(zpad, 0.0)

            # ------- main loop: software-pipelined, back-phase one block behind -------
            t_pooled = wts.tile([128, ET, QPC], F32)
            from collections import defaultdict
            partials = defaultdict(list)
            state = {}
            nt_offs = np.cumsum([0] + [2 * 128 * s for s in S_b]).tolist()

            def emit_front(b):
                sb = S_b[b]
                t_nt = big.tile([128, ET, S_MAX], FP8, tag="nt")
                nc.sync.dma_start(
                    t_nt[:, :, 0:sb],
                    bass.AP(tensor=d_nodesT.tensor, offset=nt_offs[b],
                            ap=[[sb, 128], [128 * sb, ET], [1, sb]]))
                chunks = []
                for a0 in range(0, sb, ACTW):
                    aw = min(ACTW, sb - a0)
                    zs = []
                    for dt_i in range(DT):
                        p_x = ps.tile([128, ACTW], F32, tag="mm")
                        for (p0, pw) in _pieces(aw):
                            nc.tensor.matmul(
                                p_x[:, p0 : p0 + pw],
                                sWn[:, :, dt_i * 128:(dt_i + 1) * 128],
                                t_nt[:, :, a0 + p0 : a0 + p0 + pw],
                                start=True, stop=True, perf_mode=DR)
                        t_z = zp.tile([128, ACTW], BF16, tag="z")
                        nc.scalar.activation(t_z[:, 0:aw], p_x[:, 0:aw],
                                             AF.Tanh, bias=sbn[:, dt_i : dt_i + 1],
                                             scale=1.0)
                        zs.append(t_z)
                    chunks.append((a0, aw, zs))
                state[b] = (t_nt, chunks)

            def emit_back(b):
                qi, k = b // K, b % K
                sb = S_b[b]
                t_nt, chunks = state.pop(b)

                # ---- dots via PE (gq as 1-column stationary weights) ----
                t_dots = sml.tile([1, S_MAX], F32, tag="dots")
                for (a0, aw, zs) in chunks:
                    for (p0, pw) in _pieces(aw):
                        p_d = pp.tile([128, PSW], F32, tag="pp")
                        for dt_i in range(DT):
                            nc.tensor.matmul(p_d[0:1, 0:pw],
                                             sgq[:, dt_i, qi : qi + 1],
                                             zs[dt_i][:, p0 : p0 + pw],
                                             start=(dt_i == 0),
                                             stop=(dt_i == DT - 1))
                        nc.vector.tensor_copy(t_dots[0:1, a0 + p0 : a0 + p0 + pw],
                                              p_d[0:1, 0:pw])
                nc.sync.dma_start(d_dots[b : b + 1, 0:sb], t_dots[0:1, 0:sb])

                # ---- softmax on [N, M] grid ----
                t_dbuf = sml.tile([N, M], F32, tag="dbuf")
                nc.gpsimd.tensor_scalar(t_dbuf, smaskf[:, b, :], 0.0,
                                        scq[:, qi : qi + 1],
                                        op0=ALU.mult, op1=ALU.add)
                for (L, r0, cnt, soff) in segs[b]:
                    nc.sync.dma_start(
                        t_dbuf[r0 : r0 + cnt, 0:L],
                        d_dots[b, soff : soff + cnt * L].rearrange("(c l) -> c l", l=L))

                t_gmx = sml.tile([N, 1], F32, tag="gmx")
                nc.vector.tensor_reduce(t_gmx, t_dbuf, axis=AX.X, op=ALU.max)
                t_gnmx = sml.tile([N, 1], F32, tag="gnmx")
                nc.gpsimd.tensor_scalar_mul(t_gnmx, t_gmx, -1.0)
                t_ex = sml.tile([N, M], F32, tag="ex")
                t_rs = sml.tile([N, 1], F32, tag="rs")
                nc.scalar.activation(t_ex, t_dbuf, AF.Exp, bias=t_gnmx, scale=1.0,
                                     accum_out=t_rs)
                t_ri = sml.tile([N, 1], F32, tag="ri")
                nc.vector.reciprocal(t_ri, t_rs)
                t_lg = sml.tile([N, M], F32, tag="lg")
                nc.vector.tensor_scalar(t_lg, t_ex, t_ri, spr[:, b : b + 1],
                                        op0=ALU.mult, op1=ALU.mult)
                t_gl = sml.tile([N, M], F32, tag="gl")
                t_grs = sml.tile([N, 1], F32, tag="grs")
                nc.scalar.activation(t_gl, t_lg, AF.Exp, accum_out=t_grs)
                t_gs = sml.tile([1, 1], F32, tag="gs")
                nc.gpsimd.tensor_reduce(t_gs, t_grs, axis=AX.C, op=ALU.add)
                t_gi = sml.tile([1, 1], F32, tag="gi")
                nc.vector.reciprocal(t_gi, t_gs)
                nc.sync.dma_start(d_ginv[b : b + 1, :], t_gi)
                t_gic = sml.tile([N, 1], F32, tag="gic")
                nc.sync.dma_start(
                    t_gic,
                    bass.AP(tensor=d_ginv.tensor, offset=b, ap=[[0, N], [1, 1]]))
                t_wa = sml.tile([N, M], BF16, tag="wa")
                nc.vector.scalar_tensor_tensor(
                    t_wa, t_gl, t_gic, smaskf[:, b, :],
                    op0=ALU.mult, op1=ALU.mult)

                # gather back to compacted order (+ zero the 16-pad tail)
                for (L, r0, cnt, soff) in segs[b]:
                    nc.sync.dma_start(
                        d_wa[b, soff : soff + cnt * L].rearrange("(c l) -> c l", l=L),
                        t_wa[r0 : r0 + cnt, 0:L])
                if S_raw[b] < sb:
                    nc.sync.dma_start(d_wa[b, S_raw[b] : sb],
                                      zpad[0, 0 : sb - S_raw[b]])
                t_wac = sml.tile([1, S_MAX], BF16, tag="wac")
                nc.sync.dma_start(t_wac[0:1, 0:sb], d_wa[b : b + 1, 0:sb])

                # ---- pass 2: me[e] = sum_s nodesT[e, s] * wa[s] ----
                for (p0, pw) in _pieces(sb):
                    p_w = pp.tile([128, PSW], F32, tag="pp")
                    nc.tensor.matmul(p_w[:, 0:pw], sones_r,
                                     t_wac[0:1, p0 : p0 + pw],
                                     start=True, stop=True)
                    for et in range(ET):
                        t_me = mep.tile([128, 1], F32, tag="me")
                        t_junk = jnk.tile([128, PSW], BF16, tag="junk")
                        nc.vector.scalar_tensor_tensor(
                            out=t_junk[:, 0:pw],
                            in0=t_nt[:, et, p0 : p0 + pw],
                            scalar=1.0,
                            in1=p_w[:, 0:pw],
                            op0=ALU.mult, op1=ALU.mult,
                            accum_out=t_me)
                        partials[(qi, et)].append(t_me)
                if k == K - 1:
                    for et in range(ET):
                        ps_list = partials.pop((qi, et))
                        acc_t = ps_list[0]
                        for i, t in enumerate(ps_list[1:]):
                            is_last = i == len(ps_list) - 2
                            if is_last:
                                dst = t_pooled[:, et, qi : qi + 1]
                            else:
                                dst = mep.tile([128, 1], F32, tag="macc")
                            nc.vector.tensor_tensor(dst, acc_t, t, op=ALU.add)
                            acc_t = dst

            for b in range(NB):
                emit_front(b)
                if b > 0:
                    emit_back(b - 1)
            emit_back(NB - 1)

            # ---------------- output projection ----------------
            t_plr = wts.tile([128, ET, QPC], BF16)
            nc.vector.tensor_copy(t_plr, t_pooled)
            t_outT = wts.tile([128, DT, QPC], F32)
            for mt in range(DT):
                p_o = ps.tile([128, ACTW], F32, tag="mm")
                for et in range(ET):
                    nc.tensor.matmul(p_o[:, 0:QPC],
                                     sWg[:, et, mt * 128:(mt + 1) * 128],
                                     t_plr[:, et, :],
                                     start=(et == 0), stop=(et == ET - 1))
                nc.scalar.activation(t_outT[:, mt, :], p_o[:, 0:QPC], AF.Tanh,
                                     bias=sbg[:, mt : mt + 1], scale=1.0)
            nc.sync.dma_start(d_outT.rearrange("(t p) q -> p t q", p=128), t_outT)

    nc.compile()
    return nc


_CACHE = {}


def kernel(**inputs) -> np.ndarray:
    per_core, S_b, S_raw, segs, gnn_idx, hs = _host_prep(inputs)
    key = tuple(S_b)
    if key not in _CACHE:
        _CACHE[key] = _build_program(S_b, S_raw, segs)
    nc = _CACHE[key]
    res = run_bass_kernel_spmd(nc, per_core, list(range(NCORES)))
    out = hs.copy()
    for c in range(NCORES):
        rows = res.results[c]["outT"].T      # [QPC, D]
        np.add.at(out, gnn_idx[c * QPC : (c + 1) * QPC], rows)
    return out


# revision 27
# speedup vs baseline: 1.7141x; 1.7108x over previous
"""Trainium2 Bass kernel for the CustomGNNLayer problem.

Strategy (data-parallel over Q, 8 queries/core on 8 cores):
  host: compute the tiny projection heads (rel softmax + prob gather, gq,
        c_q) in numpy; compact node slots per (q,k) group (drop all-zero
        padded slots; pad kept counts to PAD_MULT with a per-block class
        profile uniform across cores so one SPMD program fits all cores,
        then pad each block to a 16-slot multiple); nodes+Wn in fp8e4,
        everything big else bf16; fold mask / mean divisors into maskf.
  device (per core, per (q,k) block): X = Wn^T @ nodesT in one fp8
        DoubleRow matmul per (d-tile, piece); tanh+bias on ScalarE; dots
        via per-d-tile PE matmuls with gq as 1-column stationary weights,
        accumulated in PSUM; group softmax + global softmax on [N,M] grid;
        weighted sum of nodes via PE-broadcast wa + fused DVE
        multiply-reduce; final tanh projection -> updated rows.
  host: res = hidden_states.copy(); res[gnn_idx] += rows.
"""
import sys

sys.path.insert(0, "/opt/trn_rl_repo")

import numpy as np
import ml_dtypes

import concourse.bacc as bacc
import concourse.bass as bass
import concourse.tile as tile
from concourse import mybir
from concourse.bass_utils import run_bass_kernel_spmd

F32 = mybir.dt.float32
BF16 = mybir.dt.bfloat16
FP8 = mybir.dt.float8e4
AF = mybir.ActivationFunctionType
ALU = mybir.AluOpType
AX = mybir.AxisListType
DR = mybir.MatmulPerfMode.DoubleRow

Q, K, N, M = 64, 2, 32, 64
E, D, R, S = 256, 1024, 200, 8192
NCORES = 8
QPC = Q // NCORES          # 8 queries per core
NB = QPC * K               # 16 blocks per core, b = qi*K + k
PAD_MULT = 8
ET = E // 128              # 2 e-tiles
DT = D // 128              # 8 d-tiles
PSW = 512                  # psum bank width (f32)
ACTW = 3 * PSW             # activation span (3 psum banks)

BF16NP = ml_dtypes.bfloat16
FP8NP = ml_dtypes.float8_e4m3fn


def _pieces(size, step=PSW):
    return [(p0, min(step, size - p0)) for p0 in range(0, size, step)]


def _host_prep(inputs):
    hs = np.ascontiguousarray(inputs["hidden_states"], dtype=np.float32)
    nodes = np.ascontiguousarray(inputs["nodes"], dtype=np.float32)
    prob_idx = np.asarray(inputs["prob_idx"])
    gnn_idx = np.asarray(inputs["gnn_idx"]).astype(np.int64)
    rel_idx = np.asarray(inputs["rel_idx"]).astype(np.int64)
    Wc = np.asarray(inputs["Wc"], np.float32); bc = np.asarray(inputs["bc"], np.float32)
    Wq = np.asarray(inputs["Wq"], np.float32); bq = np.asarray(inputs["bq"], np.float32)
    Wn = np.asarray(inputs["Wn"], np.float32); bn = np.asarray(inputs["bn"], np.float32)
    Wg = np.asarray(inputs["Wg"], np.float32); bg = np.asarray(inputs["bg"], np.float32)

    # tiny projection heads on host
    rl = hs[rel_idx] @ Wc + bc                          # [Q,R]
    rl -= rl.max(axis=1, keepdims=True)
    np.exp(rl, out=rl)
    rel_prob = rl / rl.sum(axis=1, keepdims=True)
    probs10 = 10.0 * np.take_along_axis(
        rel_prob, prob_idx.reshape(Q, K * N), axis=1).reshape(Q, K, N)
    gq = np.tanh(hs[gnn_idx] @ Wq + bq)                 # [Q,D]
    cq = gq @ np.tanh(bn)                               # [Q]

    nz = np.any(nodes != 0.0, axis=4)          # [Q,K,N,M] kept slots
    lens = nz.sum(axis=3)                      # [Q,K,N]
    Lg = np.minimum(((np.maximum(lens, 1) + PAD_MULT - 1) // PAD_MULT) * PAD_MULT, M)

    # per-block-index profile: position-wise max of descending-sorted Lg across cores
    profiles = []   # [NB][N] descending class sizes, uniform across cores
    for qi in range(QPC):
        for k in range(K):
            seqs = [np.sort(Lg[c * QPC + qi, k])[::-1] for c in range(NCORES)]
            profiles.append(np.max(np.stack(seqs), axis=0))
    S_raw = [int(p.sum()) for p in profiles]
    S_b = [((s + 15) // 16) * 16 for s in S_raw]   # pad to 16 for fp8 APs
    segs = []       # [NB] list of (L, row0, cnt, slot_off)
    for p in profiles:
        s, off = [], 0
        i = 0
        while i < N:
            j = i
            while j < N and p[j] == p[i]:
                j += 1
            L = int(p[i])
            s.append((L, i, j - i, off))
            off += L * (j - i)
            i = j
        segs.append(s)

    mask0 = (nodes[..., 0] != 0.0)             # [Q,K,N,M] reference mask

    per_core = []
    for c in range(NCORES):
        qs = np.arange(c * QPC, (c + 1) * QPC)
        nt_flat = np.empty(sum(2 * 128 * s for s in S_b), FP8NP)
        maskf = np.zeros((NB, N, M), np.float32)
        spr = np.zeros((N, NB), np.float32)
        ntoff = 0
        for qi in range(QPC):
            q = qs[qi]
            for k in range(K):
                b = qi * K + k
                prof = profiles[b]
                order = np.argsort(-Lg[q, k], kind="stable")   # ranks -> groups
                comp = np.zeros((S_b[b], E), np.float32)
                off = 0
                for rank, g in enumerate(order):
                    L = int(prof[rank])
                    keep = np.nonzero(nz[q, k, g])[0]
                    nkeep = len(keep)
                    comp[off : off + nkeep] = nodes[q, k, g, keep]
                    maskf[b, rank, :nkeep] = mask0[q, k, g, keep].astype(np.float32)
                    spr[rank, b] = probs10[q, k, g]
                    off += L
                nt = comp.T.astype(FP8NP)                      # [E, S_b] fp8
                sz = 2 * 128 * S_b[b]
                nt_flat[ntoff : ntoff + sz] = nt.reshape(-1)
                ntoff += sz
        maskf *= 1.0 / (N * M * K)
        # gqT [128, DT, QPC]: d = t*128 + p
        gqT = np.ascontiguousarray(
            gq[qs].reshape(QPC, DT, 128).transpose(2, 1, 0)).astype(BF16NP)
        scq = np.ascontiguousarray(
            np.broadcast_to(cq[qs][None, :], (N, QPC)), np.float32)
        per_core.append({
            "nodesT": nt_flat,
            "maskf": maskf,
            "spr": spr,
            "sgq": gqT,
            "scq": scq,
        })

    shared = {
        "Wn": Wn.astype(FP8NP),
        "Wg": Wg.astype(BF16NP),
        "bn": np.ascontiguousarray(bn.reshape(DT, 128).T),
        "bg": np.ascontiguousarray(bg.reshape(DT, 128).T),
        "ones128": np.ones((1, 128), BF16NP),
    }
    for pc in per_core:
        pc.update(shared)
    return per_core, S_b, S_raw, segs, gnn_idx, hs


def _build_program(S_b, S_raw, segs):
    nc = bacc.Bacc("TRN2", target_bir_lowering=False, debug=False,
                   num_devices=NCORES)
    S_MAX = max(S_b)
    NT_TOT = sum(2 * 128 * s for s in S_b)

    d_nodesT = nc.dram_tensor("nodesT", [NT_TOT], FP8, kind="ExternalInput").ap()
    d_Wn = nc.dram_tensor("Wn", [E, D], FP8, kind="ExternalInput").ap()
    d_Wg = nc.dram_tensor("Wg", [E, D], BF16, kind="ExternalInput").ap()
    d_bn = nc.dram_tensor("bn", [128, DT], F32, kind="ExternalInput").ap()
    d_bg = nc.dram_tensor("bg", [128, DT], F32, kind="ExternalInput").ap()
    d_ones = nc.dram_tensor("ones128", [1, 128], BF16, kind="ExternalInput").ap()
    d_maskf = nc.dram_tensor("maskf", [NB, N, M], F32, kind="ExternalInput").ap()
    d_spr = nc.dram_tensor("spr", [N, NB], F32, kind="ExternalInput").ap()
    d_sgq = nc.dram_tensor("sgq", [128, DT, QPC], BF16, kind="ExternalInput").ap()
    d_scq = nc.dram_tensor("scq", [N, QPC], F32, kind="ExternalInput").ap()
    d_outT = nc.dram_tensor("outT", [D, QPC], F32, kind="ExternalOutput").ap()

    # DRAM scratch
    d_dots = nc.dram_tensor("sc_dots", [NB, 2048], F32).ap()
    d_wa = nc.dram_tensor("sc_wa", [NB, 2048], BF16).ap()
    d_ginv = nc.dram_tensor("sc_ginv", [NB, 1], F32).ap()

    with tile.TileContext(nc) as tc:
        with tc.tile_pool(name="wts", bufs=1) as wts, \
             tc.tile_pool(name="big", bufs=4) as big, \
             tc.tile_pool(name="zp", bufs=20) as zp, \
             tc.tile_pool(name="sml", bufs=6) as sml, \
             tc.tile_pool(name="mep", bufs=24) as mep, \
             tc.tile_pool(name="jnk", bufs=2) as jnk, \
             tc.tile_pool(name="ps", bufs=2, space="PSUM") as ps, \
             tc.tile_pool(name="psd", bufs=1, space="PSUM") as psd, \
             tc.tile_pool(name="psw", bufs=1, space="PSUM") as psw:

            # ---------------- load constants ----------------
            sWn = wts.tile([128, ET, D], FP8)
            nc.sync.dma_start(sWn, d_Wn.rearrange("(t p) r -> p t r", p=128))
            sWg = wts.tile([128, ET, D], BF16)
            nc.sync.dma_start(sWg, d_Wg.rearrange("(t p) r -> p t r", p=128))
            sbn = wts.tile([128, DT], F32)
            nc.sync.dma_start(sbn, d_bn)
            sbg = wts.tile([128, DT], F32)
            nc.sync.dma_start(sbg, d_bg)
            sones_r = wts.tile([1, 128], BF16)
            nc.sync.dma_start(sones_r, d_ones)
            smaskf = wts.tile([N, NB, M], F32)
            nc.sync.dma_start(smaskf, d_maskf.rearrange("b n m -> n b m"))
            spr = wts.tile([N, NB], F32)
            nc.sync.dma_start(spr, d_spr)
            sgq = wts.tile([128, DT, QPC], BF16)
            nc.sync.dma_start(sgq, d_sgq)
            scq = wts.tile([N, QPC], F32)
            nc.sync.dma_start(scq, d_scq)
            zpad = wts.tile([1, 16], BF16)
            nc.vector.memset(zpad, 0.0)

            # ------- main loop: software-pipelined, back-phase one block behind -------
            t_pooled = wts.tile([128, ET, QPC], F32)
            from collections import defaultdict
            partials = defaultdict(list)
            state = {}
            nt_offs = np.cumsum([0] + [2 * 128 * s for s in S_b]).tolist()

            def emit_front(b):
                sb = S_b[b]
                t_nt = big.tile([128, ET, S_MAX], FP8, tag="nt")
                nc.sync.dma_start(
                    t_nt[:, :, 0:sb],
                    bass.AP(tensor=d_nodesT.tensor, offset=nt_offs[b],
                            ap=[[sb, 128], [128 * sb, ET], [1, sb]]))
                chunks = []
                for a0 in range(0, sb, ACTW):
                    aw = min(ACTW, sb - a0)
                    zs = []
                    for dt_i in range(DT):
                        p_x = ps.tile([128, ACTW], F32, tag="mm")
                        for (p0, pw) in _pieces(aw):
                            nc.tensor.matmul(
                                p_x[:, p0 : p0 + pw],
                                sWn[:, :, dt_i * 128:(dt_i + 1) * 128],
                                t_nt[:, :, a0 + p0 : a0 + p0 + pw],
                                start=True, stop=True, perf_mode=DR)
                        t_z = zp.tile([128, ACTW], BF16, tag="z")
                        nc.scalar.activation(t_z[:, 0:aw], p_x[:, 0:aw],
                                             AF.Tanh, bias=sbn[:, dt_i : dt_i + 1],
                                             scale=1.0)
                        zs.append(t_z)
                    chunks.append((a0, aw, zs))
                state[b] = (t_nt, chunks)

            def emit_back(b):
                qi, k = b // K, b % K
                sb = S_b[b]
                t_nt, chunks = state.pop(b)

                # ---- dots via PE (gq as 1-column stationary weights) ----
                t_dots = sml.tile([1, S_MAX], F32, tag="dots")
                for (a0, aw, zs) in chunks:
                    for (p0, pw) in _pieces(aw):
                        p_d = psd.tile([1, PSW], F32, tag="dr")
                        for dt_i in range(DT):
                            nc.tensor.matmul(p_d[0:1, 0:pw],
                                             sgq[:, dt_i, qi : qi + 1],
                                             zs[dt_i][:, p0 : p0 + pw],
                                             start=(dt_i == 0),
                                             stop=(dt_i == DT - 1))
                        nc.vector.tensor_copy(t_dots[0:1, a0 + p0 : a0 + p0 + pw],
                                              p_d[0:1, 0:pw])
                nc.sync.dma_start(d_dots[b : b + 1, 0:sb], t_dots[0:1, 0:sb])

                # ---- softmax on [N, M] grid ----
                t_dbuf = sml.tile([N, M], F32, tag="dbuf")
                nc.gpsimd.tensor_scalar(t_dbuf, smaskf[:, b, :], 0.0,
                                        scq[:, qi : qi + 1],
                                        op0=ALU.mult, op1=ALU.add)
                for (L, r0, cnt, soff) in segs[b]:
                    nc.sync.dma_start(
                        t_dbuf[r0 : r0 + cnt, 0:L],
                        d_dots[b, soff : soff + cnt * L].rearrange("(c l) -> c l", l=L))

                t_gmx = sml.tile([N, 1], F32, tag="gmx")
                nc.vector.tensor_reduce(t_gmx, t_dbuf, axis=AX.X, op=ALU.max)
                t_gnmx = sml.tile([N, 1], F32, tag="gnmx")
                nc.gpsimd.tensor_scalar_mul(t_gnmx, t_gmx, -1.0)
                t_ex = sml.tile([N, M], F32, tag="ex")
                t_rs = sml.tile([N, 1], F32, tag="rs")
                nc.scalar.activation(t_ex, t_dbuf, AF.Exp, bias=t_gnmx, scale=1.0,
                                     accum_out=t_rs)
                t_ri = sml.tile([N, 1], F32, tag="ri")
                nc.vector.reciprocal(t_ri, t_rs)
                t_lg = sml.tile([N, M], F32, tag="lg")
                nc.vector.tensor_scalar(t_lg, t_ex, t_ri, spr[:, b : b + 1],
                                        op0=ALU.mult, op1=ALU.mult)
                t_gl = sml.tile([N, M], F32, tag="gl")
                t_grs = sml.tile([N, 1], F32, tag="grs")
                nc.scalar.activation(t_gl, t_lg, AF.Exp, accum_out=t_grs)
                t_gs = sml.tile([1, 1], F32, tag="gs")
                nc.gpsimd.tensor_reduce(t_gs, t_grs, axis=AX.C, op=ALU.add)
                t_gi = sml.tile([1, 1], F32, tag="gi")
                nc.vector.reciprocal(t_gi, t_gs)
                nc.sync.dma_start(d_ginv[b : b + 1, :], t_gi)
                t_gic = sml.tile([N, 1], F32, tag="gic")
                nc.sync.dma_start(
                    t_gic,
                    bass.AP(tensor=d_ginv.tensor, offset=b, ap=[[0, N], [1, 1]]))
                t_wa = sml.tile([N, M], BF16, tag="wa")
                nc.vector.scalar_tensor_tensor(
                    t_wa, t_gl, t_gic, smaskf[:, b, :],
                    op0=ALU.mult, op1=ALU.mult)

                # gather back to compacted order (+ zero the 16-pad tail)
                for (L, r0, cnt, soff) in segs[b]:
                    nc.sync.dma_start(
                        d_wa[b, soff : soff + cnt * L].rearrange("(c l) -> c l", l=L),
                        t_wa[r0 : r0 + cnt, 0:L])
                if S_raw[b] < sb:
                    nc.sync.dma_start(d_wa[b, S_raw[b] : sb],
                                      zpad[0, 0 : sb - S_raw[b]])
                t_wac = sml.tile([1, S_MAX], BF16, tag="wac")
                nc.sync.dma_start(t_wac[0:1, 0:sb], d_wa[b : b + 1, 0:sb])

                # ---- pass 2: me[e] = sum_s nodesT[e, s] * wa[s] ----
                for (p0, pw) in _pieces(sb):
                    p_w = psw.tile([128, PSW], F32, tag="wb")
                    nc.tensor.matmul(p_w[:, 0:pw], sones_r,
                                     t_wac[0:1, p0 : p0 + pw],
                                     start=True, stop=True)
                    for et in range(ET):
                        t_me = mep.tile([128, 1], F32, tag="me")
                        t_junk = jnk.tile([128, PSW], BF16, tag="junk")
                        nc.vector.scalar_tensor_tensor(
                            out=t_junk[:, 0:pw],
                            in0=t_nt[:, et, p0 : p0 + pw],
                            scalar=1.0,
                            in1=p_w[:, 0:pw],
                            op0=ALU.mult, op1=ALU.mult,
                            accum_out=t_me)
                        partials[(qi, et)].append(t_me)
                if k == K - 1:
                    for et in range(ET):
                        ps_list = partials.pop((qi, et))
                        acc_t = ps_list[0]
                        for i, t in enumerate(ps_list[1:]):
                            is_last = i == len(ps_list) - 2
                            if is_last:
                                dst = t_pooled[:, et, qi : qi + 1]
                            else:
                                dst = mep.tile([128, 1], F32, tag="macc")
                            nc.vector.tensor_tensor(dst, acc_t, t, op=ALU.add)
                            acc_t = dst

            for b in range(NB):
                emit_front(b)
                if b > 0:
                    emit_back(b - 1)
            emit_back(NB - 1)

            # ---------------- output projection ----------------
            t_plr = wts.tile([128, ET, QPC], BF16)
            nc.vector.tensor_copy(t_plr, t_pooled)
            t_outT = wts.tile([128, DT, QPC], F32)
            for mt in range(DT):
                p_o = ps.tile([128, ACTW], F32, tag="mm")
                for et in range(ET):
                    nc.tensor.matmul(p_o[:, 0:QPC],
                                     sWg[:, et, mt * 128:(mt + 1) * 128],
                                     t_plr[:, et, :],
                                     start=(et == 0), stop=(et == ET - 1))
                nc.scalar.activation(t_outT[:, mt, :], p_o[:, 0:QPC], AF.Tanh,
                                     bias=sbg[:, mt : mt + 1], scale=1.0)
            nc.sync.dma_start(d_outT.rearrange("(t p) q -> p t q", p=128), t_outT)

    nc.compile()
    return nc


_CACHE = {}


def kernel(**inputs) -> np.ndarray:
    per_core, S_b, S_raw, segs, gnn_idx, hs = _host_prep(inputs)
    key = tuple(S_b)
    if key not in _CACHE:
        _CACHE[key] = _build_program(S_b, S_raw, segs)
    nc = _CACHE[key]
    res = run_bass_kernel_spmd(nc, per_core, list(range(NCORES)))
    out = hs.copy()
    for c in range(NCORES):
        rows = res.results[c]["outT"].T      # [QPC, D]
        np.add.at(out, gnn_idx[c * QPC : (c + 1) * QPC], rows)
    return out


# revision 28
# speedup vs baseline: 1.7979x; 1.0489x over previous
"""Trainium2 Bass kernel for the CustomGNNLayer problem.

Strategy (data-parallel over Q, 8 queries/core on 8 cores):
  host: compute the tiny projection heads (rel softmax + prob gather, gq,
        c_q) in numpy; compact node slots per (q,k) group (drop all-zero
        padded slots; pad kept counts to PAD_MULT with a per-block class
        profile uniform across cores so one SPMD program fits all cores,
        then pad each block to a 16-slot multiple); nodes+Wn in fp8e4,
        everything big else bf16; fold mask / mean divisors into maskf.
  device (per core, per (q,k) block): X = Wn^T @ nodesT in one fp8
        DoubleRow matmul per (d-tile, piece); tanh+bias on ScalarE; dots
        via per-d-tile PE matmuls with gq as 1-column stationary weights,
        accumulated in PSUM; group softmax + global softmax on [N,M] grid;
        weighted sum of nodes via PE-broadcast wa + fused DVE
        multiply-reduce; final tanh projection -> updated rows.
  host: res = hidden_states.copy(); res[gnn_idx] += rows.
"""
import sys

sys.path.insert(0, "/opt/trn_rl_repo")

import numpy as np
import ml_dtypes

import concourse.bacc as bacc
import concourse.bass as bass
import concourse.tile as tile
from concourse import mybir
from concourse.bass_utils import run_bass_kernel_spmd

F32 = mybir.dt.float32
BF16 = mybir.dt.bfloat16
FP8 = mybir.dt.float8e4
AF = mybir.ActivationFunctionType
ALU = mybir.AluOpType
AX = mybir.AxisListType
DR = mybir.MatmulPerfMode.DoubleRow

Q, K, N, M = 64, 2, 32, 64
E, D, R, S = 256, 1024, 200, 8192
NCORES = 8
QPC = Q // NCORES          # 8 queries per core
NB = QPC * K               # 16 blocks per core, b = qi*K + k
PAD_MULT = 8
ET = E // 128              # 2 e-tiles
DT = D // 128              # 8 d-tiles
PSW = 512                  # psum bank width (f32)
ACTW = 3 * PSW             # activation span (3 psum banks)

BF16NP = ml_dtypes.bfloat16
FP8NP = ml_dtypes.float8_e4m3fn


def _pieces(size, step=PSW):
    return [(p0, min(step, size - p0)) for p0 in range(0, size, step)]


def _host_prep(inputs):
    hs = np.ascontiguousarray(inputs["hidden_states"], dtype=np.float32)
    nodes = np.ascontiguousarray(inputs["nodes"], dtype=np.float32)
    prob_idx = np.asarray(inputs["prob_idx"])
    gnn_idx = np.asarray(inputs["gnn_idx"]).astype(np.int64)
    rel_idx = np.asarray(inputs["rel_idx"]).astype(np.int64)
    Wc = np.asarray(inputs["Wc"], np.float32); bc = np.asarray(inputs["bc"], np.float32)
    Wq = np.asarray(inputs["Wq"], np.float32); bq = np.asarray(inputs["bq"], np.float32)
    Wn = np.asarray(inputs["Wn"], np.float32); bn = np.asarray(inputs["bn"], np.float32)
    Wg = np.asarray(inputs["Wg"], np.float32); bg = np.asarray(inputs["bg"], np.float32)

    # tiny projection heads on host
    rl = hs[rel_idx] @ Wc + bc                          # [Q,R]
    rl -= rl.max(axis=1, keepdims=True)
    np.exp(rl, out=rl)
    rel_prob = rl / rl.sum(axis=1, keepdims=True)
    probs10 = 10.0 * np.take_along_axis(
        rel_prob, prob_idx.reshape(Q, K * N), axis=1).reshape(Q, K, N)
    gq = np.tanh(hs[gnn_idx] @ Wq + bq)                 # [Q,D]
    cq = gq @ np.tanh(bn)                               # [Q]

    nz = np.any(nodes != 0.0, axis=4)          # [Q,K,N,M] kept slots
    lens = nz.sum(axis=3)                      # [Q,K,N]
    Lg = np.minimum(((np.maximum(lens, 1) + PAD_MULT - 1) // PAD_MULT) * PAD_MULT, M)

    # per-block-index profile: position-wise max of descending-sorted Lg across cores
    profiles = []   # [NB][N] descending class sizes, uniform across cores
    for qi in range(QPC):
        for k in range(K):
            seqs = [np.sort(Lg[c * QPC + qi, k])[::-1] for c in range(NCORES)]
            profiles.append(np.max(np.stack(seqs), axis=0))
    S_raw = [int(p.sum()) for p in profiles]
    S_b = [((s + 15) // 16) * 16 for s in S_raw]   # pad to 16 for fp8 APs
    segs = []       # [NB] list of (L, row0, cnt, slot_off)
    for p in profiles:
        s, off = [], 0
        i = 0
        while i < N:
            j = i
            while j < N and p[j] == p[i]:
                j += 1
            L = int(p[i])
            s.append((L, i, j - i, off))
            off += L * (j - i)
            i = j
        segs.append(s)

    mask0 = (nodes[..., 0] != 0.0)             # [Q,K,N,M] reference mask

    per_core = []
    for c in range(NCORES):
        qs = np.arange(c * QPC, (c + 1) * QPC)
        nt_flat = np.empty(sum(2 * 128 * s for s in S_b), FP8NP)
        maskf = np.zeros((NB, N, M), np.float32)
        spr = np.zeros((N, NB), np.float32)
        ntoff = 0
        for qi in range(QPC):
            q = qs[qi]
            for k in range(K):
                b = qi * K + k
                prof = profiles[b]
                order = np.argsort(-Lg[q, k], kind="stable")   # ranks -> groups
                comp = np.zeros((S_b[b], E), np.float32)
                off = 0
                for rank, g in enumerate(order):
                    L = int(prof[rank])
                    keep = np.nonzero(nz[q, k, g])[0]
                    nkeep = len(keep)
                    comp[off : off + nkeep] = nodes[q, k, g, keep]
                    maskf[b, rank, :nkeep] = mask0[q, k, g, keep].astype(np.float32)
                    spr[rank, b] = probs10[q, k, g]
                    off += L
                nt = comp.T.astype(FP8NP)                      # [E, S_b] fp8
                sz = 2 * 128 * S_b[b]
                nt_flat[ntoff : ntoff + sz] = nt.reshape(-1)
                ntoff += sz
        maskf *= 1.0 / (N * M * K)
        # gqT [128, DT, QPC]: d = t*128 + p
        gqT = np.ascontiguousarray(
            gq[qs].reshape(QPC, DT, 128).transpose(2, 1, 0)).astype(BF16NP)
        scq = np.ascontiguousarray(
            np.broadcast_to(cq[qs][None, :], (N, QPC)), np.float32)
        per_core.append({
            "nodesT": nt_flat,
            "maskf": maskf,
            "spr": spr,
            "sgq": gqT,
            "scq": scq,
        })

    shared = {
        "Wn": Wn.astype(FP8NP),
        "Wg": Wg.astype(BF16NP),
        "bn": np.ascontiguousarray(bn.reshape(DT, 128).T),
        "bg": np.ascontiguousarray(bg.reshape(DT, 128).T),
        "ones128": np.ones((1, 128), BF16NP),
    }
    for pc in per_core:
        pc.update(shared)
    return per_core, S_b, S_raw, segs, gnn_idx, hs


def _build_program(S_b, S_raw, segs):
    nc = bacc.Bacc("TRN2", target_bir_lowering=False, debug=False,
                   num_devices=NCORES)
    S_MAX = max(S_b)
    NT_TOT = sum(2 * 128 * s for s in S_b)

    d_nodesT = nc.dram_tensor("nodesT", [NT_TOT], FP8, kind="ExternalInput").ap()
    d_Wn = nc.dram_tensor("Wn", [E, D], FP8, kind="ExternalInput").ap()
    d_Wg = nc.dram_tensor("Wg", [E, D], BF16, kind="ExternalInput").ap()
    d_bn = nc.dram_tensor("bn", [128, DT], F32, kind="ExternalInput").ap()
    d_bg = nc.dram_tensor("bg", [128, DT], F32, kind="ExternalInput").ap()
    d_ones = nc.dram_tensor("ones128", [1, 128], BF16, kind="ExternalInput").ap()
    d_maskf = nc.dram_tensor("maskf", [NB, N, M], F32, kind="ExternalInput").ap()
    d_spr = nc.dram_tensor("spr", [N, NB], F32, kind="ExternalInput").ap()
    d_sgq = nc.dram_tensor("sgq", [128, DT, QPC], BF16, kind="ExternalInput").ap()
    d_scq = nc.dram_tensor("scq", [N, QPC], F32, kind="ExternalInput").ap()
    d_outT = nc.dram_tensor("outT", [D, QPC], F32, kind="ExternalOutput").ap()

    # DRAM scratch
    d_dots = nc.dram_tensor("sc_dots", [NB, 2048], F32).ap()
    d_wa = nc.dram_tensor("sc_wa", [NB, 2048], BF16).ap()
    d_ginv = nc.dram_tensor("sc_ginv", [NB, 1], F32).ap()

    with tile.TileContext(nc) as tc:
        with tc.tile_pool(name="wts", bufs=1) as wts, \
             tc.tile_pool(name="big", bufs=4) as big, \
             tc.tile_pool(name="zp", bufs=28) as zp, \
             tc.tile_pool(name="sml", bufs=6) as sml, \
             tc.tile_pool(name="mep", bufs=24) as mep, \
             tc.tile_pool(name="jnk", bufs=2) as jnk, \
             tc.tile_pool(name="ps", bufs=2, space="PSUM") as ps, \
             tc.tile_pool(name="psd", bufs=1, space="PSUM") as psd, \
             tc.tile_pool(name="psw", bufs=1, space="PSUM") as psw:

            # ---------------- load constants ----------------
            sWn = wts.tile([128, ET, D], FP8)
            nc.sync.dma_start(sWn, d_Wn.rearrange("(t p) r -> p t r", p=128))
            sWg = wts.tile([128, ET, D], BF16)
            nc.sync.dma_start(sWg, d_Wg.rearrange("(t p) r -> p t r", p=128))
            sbn = wts.tile([128, DT], F32)
            nc.sync.dma_start(sbn, d_bn)
            sbg = wts.tile([128, DT], F32)
            nc.sync.dma_start(sbg, d_bg)
            sones_r = wts.tile([1, 128], BF16)
            nc.sync.dma_start(sones_r, d_ones)
            smaskf = wts.tile([N, NB, M], F32)
            nc.sync.dma_start(smaskf, d_maskf.rearrange("b n m -> n b m"))
            spr = wts.tile([N, NB], F32)
            nc.sync.dma_start(spr, d_spr)
            sgq = wts.tile([128, DT, QPC], BF16)
            nc.sync.dma_start(sgq, d_sgq)
            scq = wts.tile([N, QPC], F32)
            nc.sync.dma_start(scq, d_scq)
            zpad = wts.tile([1, 16], BF16)
            nc.vector.memset(zpad, 0.0)

            # ------- main loop: software-pipelined, back-phase one block behind -------
            t_pooled = wts.tile([128, ET, QPC], F32)
            from collections import defaultdict
            partials = defaultdict(list)
            state = {}
            nt_offs = np.cumsum([0] + [2 * 128 * s for s in S_b]).tolist()

            def emit_front(b):
                sb = S_b[b]
                t_nt = big.tile([128, ET, S_MAX], FP8, tag="nt")
                nc.sync.dma_start(
                    t_nt[:, :, 0:sb],
                    bass.AP(tensor=d_nodesT.tensor, offset=nt_offs[b],
                            ap=[[sb, 128], [128 * sb, ET], [1, sb]]))
                chunks = []
                for a0 in range(0, sb, ACTW):
                    aw = min(ACTW, sb - a0)
                    zs = []
                    for dt_i in range(DT):
                        p_x = ps.tile([128, ACTW], F32, tag="mm")
                        for (p0, pw) in _pieces(aw):
                            nc.tensor.matmul(
                                p_x[:, p0 : p0 + pw],
                                sWn[:, :, dt_i * 128:(dt_i + 1) * 128],
                                t_nt[:, :, a0 + p0 : a0 + p0 + pw],
                                start=True, stop=True, perf_mode=DR)
                        t_z = zp.tile([128, ACTW], BF16, tag="z")
                        nc.scalar.activation(t_z[:, 0:aw], p_x[:, 0:aw],
                                             AF.Tanh, bias=sbn[:, dt_i : dt_i + 1],
                                             scale=1.0)
                        zs.append(t_z)
                    chunks.append((a0, aw, zs))
                state[b] = (t_nt, chunks)

            def emit_back(b):
                qi, k = b // K, b % K
                sb = S_b[b]
                t_nt, chunks = state.pop(b)

                # ---- dots via PE (gq as 1-column stationary weights) ----
                t_dots = sml.tile([1, S_MAX], F32, tag="dots")
                for (a0, aw, zs) in chunks:
                    for (p0, pw) in _pieces(aw):
                        p_d = psd.tile([1, PSW], F32, tag="dr")
                        for dt_i in range(DT):
                            nc.tensor.matmul(p_d[0:1, 0:pw],
                                             sgq[:, dt_i, qi : qi + 1],
                                             zs[dt_i][:, p0 : p0 + pw],
                                             start=(dt_i == 0),
                                             stop=(dt_i == DT - 1))
                        nc.vector.tensor_copy(t_dots[0:1, a0 + p0 : a0 + p0 + pw],
                                              p_d[0:1, 0:pw])
                nc.sync.dma_start(d_dots[b : b + 1, 0:sb], t_dots[0:1, 0:sb])

                # ---- softmax on [N, M] grid ----
                t_dbuf = sml.tile([N, M], F32, tag="dbuf")
                nc.gpsimd.tensor_scalar(t_dbuf, smaskf[:, b, :], 0.0,
                                        scq[:, qi : qi + 1],
                                        op0=ALU.mult, op1=ALU.add)
                for (L, r0, cnt, soff) in segs[b]:
                    nc.sync.dma_start(
                        t_dbuf[r0 : r0 + cnt, 0:L],
                        d_dots[b, soff : soff + cnt * L].rearrange("(c l) -> c l", l=L))

                t_gmx = sml.tile([N, 1], F32, tag="gmx")
                nc.vector.tensor_reduce(t_gmx, t_dbuf, axis=AX.X, op=ALU.max)
                t_gnmx = sml.tile([N, 1], F32, tag="gnmx")
                nc.gpsimd.tensor_scalar_mul(t_gnmx, t_gmx, -1.0)
                t_ex = sml.tile([N, M], F32, tag="ex")
                t_rs = sml.tile([N, 1], F32, tag="rs")
                nc.scalar.activation(t_ex, t_dbuf, AF.Exp, bias=t_gnmx, scale=1.0,
                                     accum_out=t_rs)
                t_ri = sml.tile([N, 1], F32, tag="ri")
                nc.vector.reciprocal(t_ri, t_rs)
                t_lg = sml.tile([N, M], F32, tag="lg")
                nc.vector.tensor_scalar(t_lg, t_ex, t_ri, spr[:, b : b + 1],
                                        op0=ALU.mult, op1=ALU.mult)
                t_gl = sml.tile([N, M], F32, tag="gl")
                t_grs = sml.tile([N, 1], F32, tag="grs")
                nc.scalar.activation(t_gl, t_lg, AF.Exp, accum_out=t_grs)
                t_gs = sml.tile([1, 1], F32, tag="gs")
                nc.gpsimd.tensor_reduce(t_gs, t_grs, axis=AX.C, op=ALU.add)
                t_gi = sml.tile([1, 1], F32, tag="gi")
                nc.vector.reciprocal(t_gi, t_gs)
                # fold 1/gs into the ws broadcast weights: gi128 = gi * ones[1,128]
                t_gi128 = sml.tile([1, 128], BF16, tag="gi128")
                nc.vector.tensor_scalar_mul(t_gi128, sones_r, t_gi)
                t_wa = sml.tile([N, M], BF16, tag="wa")
                nc.vector.tensor_tensor(t_wa, t_gl, smaskf[:, b, :], op=ALU.mult)

                # gather back to compacted order (+ zero the 16-pad tail)
                for (L, r0, cnt, soff) in segs[b]:
                    nc.sync.dma_start(
                        d_wa[b, soff : soff + cnt * L].rearrange("(c l) -> c l", l=L),
                        t_wa[r0 : r0 + cnt, 0:L])
                if S_raw[b] < sb:
                    nc.sync.dma_start(d_wa[b, S_raw[b] : sb],
                                      zpad[0, 0 : sb - S_raw[b]])
                t_wac = sml.tile([1, S_MAX], BF16, tag="wac")
                nc.sync.dma_start(t_wac[0:1, 0:sb], d_wa[b : b + 1, 0:sb])

                # ---- pass 2: me[e] = sum_s nodesT[e, s] * wa[s] ----
                for (p0, pw) in _pieces(sb):
                    p_w = psw.tile([128, PSW], F32, tag="wb")
                    nc.tensor.matmul(p_w[:, 0:pw], t_gi128,
                                     t_wac[0:1, p0 : p0 + pw],
                                     start=True, stop=True)
                    for et in range(ET):
                        t_me = mep.tile([128, 1], F32, tag="me")
                        t_junk = jnk.tile([128, PSW], BF16, tag="junk")
                        nc.vector.scalar_tensor_tensor(
                            out=t_junk[:, 0:pw],
                            in0=t_nt[:, et, p0 : p0 + pw],
                            scalar=1.0,
                            in1=p_w[:, 0:pw],
                            op0=ALU.mult, op1=ALU.mult,
                            accum_out=t_me)
                        partials[(qi, et)].append(t_me)
                if k == K - 1:
                    for et in range(ET):
                        ps_list = partials.pop((qi, et))
                        acc_t = ps_list[0]
                        for i, t in enumerate(ps_list[1:]):
                            is_last = i == len(ps_list) - 2
                            if is_last:
                                dst = t_pooled[:, et, qi : qi + 1]
                            else:
                                dst = mep.tile([128, 1], F32, tag="macc")
                            nc.vector.tensor_tensor(dst, acc_t, t, op=ALU.add)
                            acc_t = dst

            for b in range(NB):
                emit_front(b)
                if b > 0:
                    emit_back(b - 1)
            emit_back(NB - 1)

            # ---------------- output projection ----------------
            t_plr = wts.tile([128, ET, QPC], BF16)
            nc.vector.tensor_copy(t_plr, t_pooled)
            t_outT = wts.tile([128, DT, QPC], F32)
            for mt in range(DT):
                p_o = ps.tile([128, ACTW], F32, tag="mm")
                for et in range(ET):
                    nc.tensor.matmul(p_o[:, 0:QPC],
                                     sWg[:, et, mt * 128:(mt + 1) * 128],
                                     t_plr[:, et, :],
                                     start=(et == 0), stop=(et == ET - 1))
                nc.scalar.activation(t_outT[:, mt, :], p_o[:, 0:QPC], AF.Tanh,
                                     bias=sbg[:, mt : mt + 1], scale=1.0)
            nc.sync.dma_start(d_outT.rearrange("(t p) q -> p t q", p=128), t_outT)

    nc.compile()
    return nc


_CACHE = {}


def kernel(**inputs) -> np.ndarray:
    per_core, S_b, S_raw, segs, gnn_idx, hs = _host_prep(inputs)
    key = tuple(S_b)
    if key not in _CACHE:
        _CACHE[key] = _build_program(S_b, S_raw, segs)
    nc = _CACHE[key]
    res = run_bass_kernel_spmd(nc, per_core, list(range(NCORES)))
    out = hs.copy()
    for c in range(NCORES):
        rows = res.results[c]["outT"].T      # [QPC, D]
        np.add.at(out, gnn_idx[c * QPC : (c + 1) * QPC], rows)
    return out
